# revision 11
# baseline (speedup 1.0000x reference)
"""BiCutLoss Trainium2 kernel (nn_BiCutLoss_52312701665760).

Reference computation (per batch row i of output[B, L, 2], labels[B, L]):
  temp = argmax(output, -1)            # 1 iff out1 > out0
  cut  = L if all(temp == 1) else (index of last 0 in temp)
  mask = arange(L) < cut
  r1   = where(labels == 1, -3.6/log2(j+2), 0.065)
  loss = sum(out1 * mask * r1) / B

Kernel formulation (equivalent):
  d[j] = out0[j] - out1[j]                       # temp[j]==0  <=>  d[j] >= 0
  M[j] = max(d[j:], -1)  (reverse cummax; M[L] = -1 pad)
  thr  = 0 if M[0] >= 0 else -BIG                # all-ones row => mask all 1
  mask[j] = (M[j+1] >= thr)
  v[j] = out1[j] * (lab[j]*(r1pos[j] - 0.065) + 0.065)   # t1 * r1, unmasked
  loss_i = sum_j mask[j] * v[j]

Sharding: pure data parallel — B=4096 rows split as 512 rows x 8 cores; each
core computes per-row partials [128,1] (4 row-tiles of 128 partitions), host
sums and divides by B.

Schedule (v2): DMA-bound design. Each [128, 4096] row-tile is processed in
NCH=4 column chunks of 1024 so compute starts as soon as the first out-chunk
DMA lands (out chunks are issued high-to-low to feed the right-to-left
suffix-max scan, chained across chunks via initial=M[chunk boundary]).
Engine balance per row-tile (cost model, ns):
  DVE : sub c3+c0 (2x1082) + scan (4x1082) + thr + lp=lab*pre2 all-bf16 TT
        (2194, 2x mode) + 4 chunk STT mask-mult-accum (4x1082)  ~= 13.2us
  Pool: sub c2+c1 (2x2056) + v = t1 * rr chunks (4x2056) + swdge  ~= 13.4us
  ACT : rr = lp + 0.065 (bf16 in, f32 out; 3598)
  DMA : 16.8 MB out + 8.4 MB lab(int32, swdge-cast to bf16) + 1 MB pre(bf16)
        ~= 17.5us per row-tile -> the binding resource.
labels are cast int32->bf16 during DMA (SWDGE); pre2 is host-precomputed in
bf16 (exact products with 0/1 labels; +0.065 restored in f32 on ACT).
"""

import os
from contextlib import ExitStack

import numpy as np

B, L = 4096, 4096
N_CORES = 8
ROWS_PER_CORE = B // N_CORES          # 512
P = 128                               # partitions per tile
TILES = ROWS_PER_CORE // P            # 4
NCH = 4                               # column chunks per row-tile
CH = L // NCH                         # 1024
C_CONST = 0.65 * 0.1                  # 0.065
BIG = 1e30

LAB_BF16 = True                       # SWDGE int32->bf16 cast; False => f32

_CACHE = {}
NAMES = {}


def _lbl(inst, s):
    try:
        NAMES[inst.ins.name] = s
    except Exception:
        pass
    return inst


def _build_nc(repeat: int = 1):
    import concourse.mybir as mybir
    import concourse.tile as tile
    from concourse import bacc

    f32 = mybir.dt.float32
    bf16 = mybir.dt.bfloat16
    i32 = mybir.dt.int32
    Op = mybir.AluOpType
    lab_dt = bf16 if LAB_BF16 else f32

    nc = bacc.Bacc("TRN2", target_bir_lowering=False, debug=False)

    out_d = nc.dram_tensor("out", [ROWS_PER_CORE, L * 2], f32, kind="ExternalInput")
    lab_d = nc.dram_tensor("lab", [ROWS_PER_CORE, L], i32, kind="ExternalInput")
    pre_d = nc.dram_tensor("pre", [P, L], bf16, kind="ExternalInput")
    res_d = nc.dram_tensor("res", [P, 1], f32, kind="ExternalOutput")

    out_t = out_d[:].rearrange("(n p) m -> n p m", p=P)   # [4, 128, 8192]
    lab_t = lab_d[:].rearrange("(n p) m -> n p m", p=P)   # [4, 128, 4096]

    with tile.TileContext(nc) as tc, ExitStack() as ctx:
        io_pool = ctx.enter_context(tc.tile_pool(name="io", bufs=2))
        pre_pool = ctx.enter_context(tc.tile_pool(name="pre", bufs=1))
        d_pool = ctx.enter_context(tc.tile_pool(name="d", bufs=1))
        m_pool = ctx.enter_context(tc.tile_pool(name="m", bufs=2))
        lp_pool = ctx.enter_context(tc.tile_pool(name="lp", bufs=2))
        rr_pool = ctx.enter_context(tc.tile_pool(name="rr", bufs=2))
        v_pool = ctx.enter_context(tc.tile_pool(name="v", bufs=2))
        vcp_pool = ctx.enter_context(tc.tile_pool(name="vcp", bufs=2))
        acc_pool = ctx.enter_context(tc.tile_pool(name="acc", bufs=1))

        pre_tl = pre_pool.tile([P, L], bf16)
        acc_B = acc_pool.tile([P, TILES * NCH], f32)
        acc_T = acc_pool.tile([P, TILES * NCH], f32, tag="accT")
        flags = acc_pool.tile([P, TILES], f32, tag="flags")

        for _r in range(repeat):
            for k in range(TILES):
                # ---- DMAs: labels first (so lp->rr is ready before the out
                # chunks land), then out chunks high->low (feeds the
                # right-to-left scan earliest).
                lt = io_pool.tile([P, L], lab_dt, tag="lt")
                _lbl(nc.gpsimd.dma_start(lt[:], lab_t[k]), f"dma_lab{k}")
                if _r == 0 and k == 0:
                    nc.sync.dma_start(pre_tl[:], pre_d[:])
                ot_ch = []
                for c in range(NCH - 1, -1, -1):
                    oc = io_pool.tile([P, 2 * CH], f32, tag=f"ot{c}")
                    _lbl(nc.sync.dma_start(
                        oc[:], out_t[k][:, c * 2 * CH:(c + 1) * 2 * CH]), f"dma_out{k}c{c}")
                    ot_ch.append((c, oc))

                # ---- pass 1+2: d chunks (DVE/Pool split) + chained scan
                M = m_pool.tile([P, L + 1], bf16)
                nc.vector.memset(M[:, L:L + 1], -1.0)
                for c, oc in ot_ch:            # high -> low
                    x3 = oc[:].rearrange("p (l two) -> p l two", two=2)
                    t0c = x3[:, :, 0]
                    t1c = x3[:, :, 1]
                    dch = d_pool.tile([P, CH], bf16, tag=f"d{c}")
                    eng = nc.vector if c in (NCH - 1, 0) else nc.gpsimd
                    _lbl(eng.tensor_tensor(dch[:], t0c, t1c, Op.subtract), f"sub{k}c{c}")
                    if c == NCH - 1:
                        ini = -1.0
                    else:
                        ini = M[:, (c + 1) * CH:(c + 1) * CH + 1]
                    _lbl(nc.vector.tensor_tensor_scan(
                        M[:, c * CH:(c + 1) * CH][:, ::-1],
                        dch[:, ::-1], dch[:, ::-1], ini,
                        Op.max, Op.max,
                    ), f"scan{k}c{c}")

                # ---- lp = lab * pre2 (all-bf16 TT on DVE: 2x mode)
                # tile_wait_until keeps the scheduler from hoisting lp into an
                # earlier tile's DVE stream (the in-order DVE sequencer would
                # then stall on the label DMA while scans sit ready).
                lp = lp_pool.tile([P, L], bf16)
                rr = rr_pool.tile([P, L], f32)
                with tc.tile_wait_until((_r * TILES + k) * 0.0146):
                    _lbl(nc.vector.tensor_tensor(lp[:], lt[:], pre_tl[:], Op.mult), f"lp{k}")
                    # ---- rr = lp + 0.065 (ACT, f32 out)
                    _lbl(nc.scalar.activation(
                        rr[:], lp[:], mybir.ActivationFunctionType.Copy,
                        bias=C_CONST, scale=1.0), f"rr{k}")

                # ---- v chunks (Pool) + two accumulations per chunk:
                #   P (DVE STT): masked with threshold 0 (normal-row mask)
                #   T (ACT copy+accum): unmasked sum (all-ones-row loss)
                # The all-ones special case is resolved per row at the end via
                # flag = (M[0] >= 0), removing the global thr dependency so
                # chunk STTs pipeline right behind their scan.
                v = v_pool.tile([P, L], f32)
                for c, oc in ot_ch:
                    cs, ce = c * CH, (c + 1) * CH
                    x3 = oc[:].rearrange("p (l two) -> p l two", two=2)
                    t1c = x3[:, :, 1]
                    _lbl(nc.gpsimd.tensor_tensor(
                        v[:, cs:ce], t1c, rr[:, cs:ce], Op.mult), f"v{k}c{c}")
                    vc = vcp_pool.tile([P, CH], f32, tag="vc")
                    _lbl(nc.scalar.activation(
                        vc[:], v[:, cs:ce], mybir.ActivationFunctionType.Copy,
                        bias=0.0, scale=1.0,
                        accum_out=acc_T[:, k * NCH + c:k * NCH + c + 1],
                    ), f"vcopy{k}c{c}")
                    # in-place masked mult; accum -> per-chunk partial sum
                    _lbl(nc.vector.scalar_tensor_tensor(
                        v[:, cs:ce], M[:, cs + 1:ce + 1], 0.0, v[:, cs:ce],
                        Op.is_ge, Op.mult,
                        accum_out=acc_B[:, k * NCH + c:k * NCH + c + 1],
                    ), f"stt{k}c{c}")

                # flag_k = 1 if row has any d >= 0 (normal), else 0
                nc.vector.tensor_scalar(
                    flags[:, k:k + 1], M[:, 0:1], 0.0, 1.0, Op.is_ge, Op.mult)

            # tail: per-tile P/T sums, then select by flag and sum tiles
            lossK = acc_pool.tile([P, TILES], f32, tag="lossK")
            for k in range(TILES):
                pk = acc_pool.tile([P, 1], f32, tag="pk")
                tk = acc_pool.tile([P, 1], f32, tag="tk")
                nc.vector.reduce_sum(
                    pk[:], acc_B[:, k * NCH:(k + 1) * NCH],
                    axis=mybir.AxisListType.X)
                nc.vector.reduce_sum(
                    tk[:], acc_T[:, k * NCH:(k + 1) * NCH],
                    axis=mybir.AxisListType.X)
                # lossK = flag*pk + (1-flag)*tk = (pk - tk)*flag + tk
                nc.vector.tensor_tensor(pk[:], pk[:], tk[:], Op.subtract)
                nc.vector.scalar_tensor_tensor(
                    lossK[:, k:k + 1], pk[:], flags[:, k:k + 1], tk[:],
                    Op.mult, Op.add)
            loss_t = acc_pool.tile([P, 1], f32, tag="loss")
            nc.vector.reduce_sum(loss_t[:], lossK[:], axis=mybir.AxisListType.X)

        nc.sync.dma_start(res_d[:], loss_t[:])

    nc.compile()
    return nc


def _pre_tile() -> np.ndarray:
    import ml_dtypes

    j = np.arange(L, dtype=np.float64)
    pre2 = (-3.6 / np.log2(j + 2.0) - C_CONST).astype(ml_dtypes.bfloat16)
    return np.ascontiguousarray(np.tile(pre2[None, :], (P, 1)))


def _get_nc(repeat: int = 1):
    key = repeat
    if key not in _CACHE:
        _CACHE[key] = _build_nc(repeat=repeat)
    return _CACHE[key]


def make_in_maps(output: np.ndarray, labels: np.ndarray):
    pre = _pre_tile()
    in_maps = []
    for c in range(N_CORES):
        sl = slice(c * ROWS_PER_CORE, (c + 1) * ROWS_PER_CORE)
        in_maps.append({
            "out": np.ascontiguousarray(output[sl]).reshape(ROWS_PER_CORE, L * 2),
            "lab": np.ascontiguousarray(labels[sl]),
            "pre": pre,
        })
    return in_maps


def kernel(output: np.ndarray, labels: np.ndarray) -> np.ndarray:
    from concourse.bass_utils import run_bass_kernel_spmd

    nc = _get_nc(repeat=1)
    in_maps = make_in_maps(output, labels)
    r = run_bass_kernel_spmd(nc, in_maps, core_ids=list(range(N_CORES)))
    total = 0.0
    for res in r.results:
        total += float(res["res"].astype(np.float64).sum())
    return np.float32(total / B)


if __name__ == "__main__":
    # quick standalone run (full inputs, random)
    rng = np.random.default_rng(0)
    out = rng.standard_normal((B, L, 2)).astype(np.float32)
    lab = rng.integers(0, 2, size=(B, L)).astype(np.int32)
    print("loss:", kernel(out, lab))
